# revision 8
# baseline (speedup 1.0000x reference)
"""Masked dot-product attention (B=2,H=16,L=2048,D=128) on 8 trn2 NeuronCores.

Strategy (v2):
  - Shard batch*heads: core c handles (b=0,h=2c),(0,2c+1),(1,2c),(1,2c+1)
    -> 4 slots, so every core carries one K0-slot pair and one K1-slot pair
    (balanced work).
  - Host pre-transposes q and k per slot into [D, L] / [D, Kv*128] bf16 and
    pre-permutes v into its SBUF image [128, Kv, 130] bf16 with a ones column
    at d=128 -> zero on-device transposes, fully contiguous DMAs.
  - Scores: S^T[k, q] = matmul(lhsT=kT_j, rhs=qT-block) in bf16
    (1 cycle/row).  Masking is a per-partition bias on the exp of the last
    key tile only.
  - exp fused into PSUM->SBUF eviction on the Act engine with
    scale=1/sqrt(D), j-pairs share one instruction; pT evicted as bf16.
  - PV: O[q, d] computed in natural layout via
    matmul(out[128q, 129], lhsT=pT[:, jj, qtile], rhs=[V_j | ones]);
    the appended ones column accumulates the softmax denominator l per
    q-partition for free (no [1,N] l-matmul, no transposes of O or l).
  - Finish per q-block: DVE reciprocal of the l column + 4 per-partition
    scalar muls, then one contiguous 256KB DMA of the fp32 output.
  - Software pipelining: scores for group g+1 are emitted before PV of
    group g so the in-order PE queue never head-of-line blocks on the Act
    engine; st/pT/o_ps pools are double-buffered (8 PSUM banks exactly).
"""

import math

import numpy as np

try:
    import concourse.bass as bass
except ImportError:  # pragma: no cover
    import sys

    sys.path.append("/opt/trn_rl_repo")
    import concourse.bass as bass

import ml_dtypes
import concourse.mybir as mybir
import concourse.tile as tile
from concourse import bacc
from concourse.bass_utils import run_bass_kernel_spmd

B, H, L, D = 2, 16, 2048, 128
NCORES = 8
HPC = H // NCORES  # heads per core per batch
SLOTS = B * HPC  # bh slots per core
NEG = -1e9
INV_SQRT_D = 1.0 / math.sqrt(D)
F32 = mybir.dt.float32
BF16 = mybir.dt.bfloat16
QB = 4  # q blocks per slot
QBW = L // QB  # 512 q per block
QTB = QBW // 128  # 4 q tiles per block
VW = 130  # v tile width: 128 d + ones col + pad
EXPF = mybir.ActivationFunctionType.Exp
NPBF16 = np.dtype(ml_dtypes.bfloat16)

_cache: dict = {}


def _jgroups(Kv):
    """j in pairs, the last j always alone (it takes the mask bias)."""
    out = []
    j = 0
    while j < Kv - 1:
        n = 2 if j + 2 <= Kv - 1 else 1
        out.append((j, n))
        j += n
    out.append((Kv - 1, 1))
    return out


def _build(K0: int, K1: int):
    """Build+compile the per-core program for K0/K1 valid key tiles."""
    Ks = [K0, K0, K1, K1]
    KM = max(K0, K1)
    nc = bacc.Bacc("TRN2", target_bir_lowering=False, debug=False, num_devices=NCORES)
    qT = nc.dram_tensor("qT", [SLOTS, 128, L], BF16, kind="ExternalInput")
    kT = nc.dram_tensor("kT", [SLOTS, 128, KM * 128], BF16, kind="ExternalInput")
    vp = nc.dram_tensor("vp", [SLOTS, 128, KM * VW], BF16, kind="ExternalInput")
    biases = nc.dram_tensor("biases", [128, SLOTS], F32, kind="ExternalInput")
    out = nc.dram_tensor("out", [SLOTS, L, D], F32, kind="ExternalOutput")

    order = sorted(range(SLOTS), key=lambda x: -Ks[x])

    with tile.TileContext(nc) as tc:
        with (
            tc.tile_pool(name="const", bufs=1) as constp,
            tc.tile_pool(name="io", bufs=1) as iop,
            tc.tile_pool(name="pt", bufs=4) as ptp,
            tc.tile_pool(name="fin", bufs=3) as finp,
            tc.tile_pool(name="psst", bufs=2, space="PSUM") as psst,
            tc.tile_pool(name="psoa", bufs=1, space="PSUM") as psoa,
        ):
            bias_sb = constp.tile([128, SLOTS], F32)
            nc.sync.dma_start(out=bias_sb, in_=biases[:, :])

            # preload all slot inputs upfront (SBUF easily fits them);
            # k/v first for the first slot so compute can start early,
            # q in per-block chunks so the first matmul doesn't wait on
            # the whole 512KB load
            kts, vps, qts = {}, {}, {}
            s0 = order[0]
            kts[s0] = constp.tile([128, KM * 128], BF16, tag=f"kt{s0}", name=f"kt{s0}")
            nc.sync.dma_start(out=kts[s0][:, : Ks[s0] * 128], in_=kT[s0, :, : Ks[s0] * 128])
            vps[s0] = constp.tile([128, KM, VW], BF16, tag=f"vp{s0}", name=f"vp{s0}")
            nc.sync.dma_start(
                out=vps[s0][:, : Ks[s0], :],
                in_=vp[s0, :, : Ks[s0] * VW].rearrange("p (t w) -> p t w", w=VW),
            )
            qts[s0] = constp.tile([128, L], BF16, tag=f"qt{s0}", name=f"qt{s0}")
            for qb in range(QB):
                nc.sync.dma_start(
                    out=qts[s0][:, qb * QBW : (qb + 1) * QBW],
                    in_=qT[s0, :, qb * QBW : (qb + 1) * QBW],
                )
            for s in order[1:]:
                Kv = Ks[s]
                kts[s] = constp.tile([128, KM * 128], BF16, tag=f"kt{s}", name=f"kt{s}")
                nc.sync.dma_start(out=kts[s][:, : Kv * 128], in_=kT[s, :, : Kv * 128])
                vps[s] = constp.tile([128, KM, VW], BF16, tag=f"vp{s}", name=f"vp{s}")
                nc.sync.dma_start(
                    out=vps[s][:, :Kv, :],
                    in_=vp[s, :, : Kv * VW].rearrange("p (t w) -> p t w", w=VW),
                )
                qts[s] = constp.tile([128, L], BF16, tag=f"qt{s}", name=f"qt{s}")
                nc.sync.dma_start(out=qts[s], in_=qT[s])

            # flat work list: one unit per (slot, q-block); scores for the
            # next unit's first group are emitted during the current unit's
            # second-to-last group so the Act engine never drains at unit
            # boundaries (the st pool's 2-buffer rotation already allows it)
            units = [(s, qb) for s in order for qb in range(QB)]

            def emit_scores(s, qb, g):
                j0, npair = _jgroups(Ks[s])[g]
                qs = qts[s][:, qb * QBW : (qb + 1) * QBW]
                st = psst.tile([128, npair, QBW], F32, tag="st", name="st")
                for jj in range(npair):
                    nc.tensor.matmul(
                        st[:, jj, :],
                        kts[s][:, (j0 + jj) * 128 : (j0 + jj + 1) * 128],
                        qs,
                        start=True,
                        stop=True,
                    )
                return st

            st_next = None
            for i, (s, qb) in enumerate(units):
                Kv = Ks[s]
                groups = _jgroups(Kv)
                G = len(groups)
                vp_sb = vps[s]
                sts = {0: st_next if st_next is not None else emit_scores(s, qb, 0)}
                st_next = None
                # one PSUM bank per q-tile accumulator (the PE cannot
                # interleave two accumulation regions within one bank);
                # col 128 accumulates l via the ones column of vp
                o_ps = psoa.tile([128, QTB, 512], F32, tag="o_ps")

                for g, (j0, npair) in enumerate(groups):
                    st = sts[g]
                    last = j0 + npair == Kv
                    pT = ptp.tile([128, npair, QBW], BF16, tag="pT")
                    nc.scalar.activation(
                        pT,
                        st[:, :npair, :],
                        EXPF,
                        bias=(bias_sb[:, s : s + 1] if last else 0.0),
                        scale=INV_SQRT_D,
                    )
                    if g + 1 < G:
                        sts[g + 1] = emit_scores(s, qb, g + 1)
                    if g == G - 2 and i + 1 < len(units):
                        st_next = emit_scores(*units[i + 1], 0)
                    for jj in range(npair):
                        j = j0 + jj
                        for qt in range(QTB):
                            nc.tensor.matmul(
                                o_ps[:, qt, : D + 1],
                                pT[:, jj, qt * 128 : (qt + 1) * 128],
                                vp_sb[:, j, : D + 1],
                                start=(j == 0),
                                stop=(j == Kv - 1),
                                skip_group_check=True,
                            )

                # fine-grained finish: free each o_ps bank as soon as its
                # q-tile's accumulation stops, so the next unit's PV matmuls
                # overlap with this finish
                lrec = finp.tile([128, QTB], F32, tag="lrec")
                o_sb = finp.tile([128, QTB, 128], F32, tag="o_sb")
                for qt in range(QTB):
                    nc.vector.reciprocal(lrec[:, qt : qt + 1], o_ps[:, qt, D : D + 1])
                    nc.vector.tensor_scalar_mul(
                        o_sb[:, qt, :],
                        o_ps[:, qt, :D],
                        lrec[:, qt : qt + 1],
                    )
                nc.sync.dma_start(
                    out=out[s].rearrange("(b t p) d -> p b t d", p=128, t=QTB)[:, qb],
                    in_=o_sb,
                )
    nc.compile()
    return nc


def _get_program(K0: int, K1: int):
    key = (K0, K1)
    if key not in _cache:
        _cache[key] = _build(K0, K1)
    return _cache[key]


def _run(q, k, v, valid_lens, trace=False):
    q = np.asarray(q, dtype=np.float32)
    k = np.asarray(k, dtype=np.float32)
    v = np.asarray(v, dtype=np.float32)
    vl = np.asarray(valid_lens).astype(np.int64)
    K0 = int(max(1, -(-vl[0] // 128)))
    K1 = int(max(1, -(-vl[1] // 128)))
    KM = max(K0, K1)
    nc = _get_program(K0, K1)

    # per-slot mask bias column: 0 for valid positions in the last key tile,
    # -1e9 beyond valid_len
    biases = np.zeros((128, SLOTS), dtype=np.float32)
    Ks = [K0, K0, K1, K1]
    bs = [0, 0, 1, 1]
    pos = np.arange(128)
    for s in range(SLOTS):
        rem = int(vl[bs[s]]) - (Ks[s] - 1) * 128
        biases[:, s] = np.where(pos < rem, 0.0, np.float32(NEG))

    # host-side prep: [B,H,L,D] fp32 -> per-slot transposed bf16 images
    qb16 = q.astype(NPBF16)  # [B,H,L,D]
    kb16 = k.astype(NPBF16)
    vb16 = v.astype(NPBF16)

    in_maps = []
    for c in range(NCORES):
        h0, h1 = 2 * c, 2 * c + 1
        bh = [(0, h0), (0, h1), (1, h0), (1, h1)]
        qTs = np.empty((SLOTS, 128, L), dtype=NPBF16)
        kTs = np.zeros((SLOTS, 128, KM * 128), dtype=NPBF16)
        vps = np.zeros((SLOTS, 128, KM * VW), dtype=NPBF16)
        for s, (b, h) in enumerate(bh):
            qTs[s] = qb16[b, h].T
            Kv = Ks[s]
            kTs[s, :, : Kv * 128] = kb16[b, h, : Kv * 128].T
            # v SBUF image: [p, t, w]: w<128 -> v[t*128+p, w]; w==128 -> 1
            vt = np.zeros((128, Kv, VW), dtype=NPBF16)
            vt[:, :, :128] = vb16[b, h, : Kv * 128].reshape(Kv, 128, 128).transpose(
                1, 0, 2
            )
            vt[:, :, 128] = NPBF16.type(1.0)
            vps[s, :, : Kv * VW] = vt.reshape(128, Kv * VW)
        in_maps.append(
            {
                "qT": np.ascontiguousarray(qTs),
                "kT": np.ascontiguousarray(kTs),
                "vp": np.ascontiguousarray(vps),
                "biases": biases,
            }
        )

    try:
        res = run_bass_kernel_spmd(
            nc, in_maps, core_ids=list(range(NCORES)), trace=trace
        )
    except Exception:
        # transient device wedges (NRT_EXEC_UNIT_UNRECOVERABLE) have been
        # observed to clear on retry
        res = run_bass_kernel_spmd(
            nc, in_maps, core_ids=list(range(NCORES)), trace=trace
        )

    outp = np.empty((B, H, L, D), dtype=np.float32)
    for c in range(NCORES):
        o = res.results[c]["out"]
        h0, h1 = 2 * c, 2 * c + 1
        outp[0, h0] = o[0]
        outp[0, h1] = o[1]
        outp[1, h0] = o[2]
        outp[1, h1] = o[3]
    return outp, res


def kernel(q, k, v, valid_lens):
    outp, _ = _run(q, k, v, valid_lens, trace=False)
    return outp


# revision 10
# speedup vs baseline: 1.5158x; 1.5158x over previous
"""Masked dot-product attention (B=2,H=16,L=2048,D=128) on 8 trn2 NeuronCores.

Strategy (v2):
  - Shard batch*heads: core c handles (b=0,h=2c),(0,2c+1),(1,2c),(1,2c+1)
    -> 4 slots, so every core carries one K0-slot pair and one K1-slot pair
    (balanced work).
  - Host pre-transposes q and k per slot into [D, L] / [D, Kv*128] bf16 and
    pre-permutes v into its SBUF image [128, Kv, 130] bf16 with a ones column
    at d=128 -> zero on-device transposes, fully contiguous DMAs.
  - Scores: S^T[k, q] = matmul(lhsT=kT_j, rhs=qT-block) in bf16
    (1 cycle/row).  Masking is a per-partition bias on the exp of the last
    key tile only.
  - exp fused into PSUM->SBUF eviction on the Act engine with
    scale=1/sqrt(D), j-pairs share one instruction; pT evicted as bf16.
  - PV: O[q, d] computed in natural layout via
    matmul(out[128q, 129], lhsT=pT[:, jj, qtile], rhs=[V_j | ones]);
    the appended ones column accumulates the softmax denominator l per
    q-partition for free (no [1,N] l-matmul, no transposes of O or l).
  - Finish per q-block: DVE reciprocal of the l column + 4 per-partition
    scalar muls, then one contiguous 256KB DMA of the fp32 output.
  - Software pipelining: scores for group g+1 are emitted before PV of
    group g so the in-order PE queue never head-of-line blocks on the Act
    engine; st/pT/o_ps pools are double-buffered (8 PSUM banks exactly).
"""

import math

import numpy as np

try:
    import concourse.bass as bass
except ImportError:  # pragma: no cover
    import sys

    sys.path.append("/opt/trn_rl_repo")
    import concourse.bass as bass

import ml_dtypes
import concourse.mybir as mybir
import concourse.tile as tile
from concourse import bacc
from concourse.bass_utils import run_bass_kernel_spmd

B, H, L, D = 2, 16, 2048, 128
NCORES = 8
HPC = H // NCORES  # heads per core per batch
SLOTS = B * HPC  # bh slots per core
NEG = -1e9
INV_SQRT_D = 1.0 / math.sqrt(D)
F32 = mybir.dt.float32
BF16 = mybir.dt.bfloat16
QB = 4  # q blocks per slot
QBW = L // QB  # 512 q per block
QTB = QBW // 128  # 4 q tiles per block
VW = 130  # v tile width: 128 d + ones col + pad
EXPF = mybir.ActivationFunctionType.Exp
NPBF16 = np.dtype(ml_dtypes.bfloat16)

_cache: dict = {}


def _jgroups(Kv):
    """j in pairs, the last j always alone (it takes the mask bias)."""
    out = []
    j = 0
    while j < Kv - 1:
        n = 2 if j + 2 <= Kv - 1 else 1
        out.append((j, n))
        j += n
    out.append((Kv - 1, 1))
    return out


def _build(K0: int, K1: int):
    """Build+compile the per-core program for K0/K1 valid key tiles."""
    Ks = [K0, K0, K1, K1]
    KM = max(K0, K1)
    nc = bacc.Bacc("TRN2", target_bir_lowering=False, debug=False, num_devices=NCORES)
    qT = nc.dram_tensor("qT", [SLOTS, 128, L], BF16, kind="ExternalInput")
    kT = nc.dram_tensor("kT", [SLOTS, 128, KM * 128], BF16, kind="ExternalInput")
    vp = nc.dram_tensor("vp", [SLOTS, 128, KM * VW], BF16, kind="ExternalInput")
    biases = nc.dram_tensor("biases", [128, SLOTS], F32, kind="ExternalInput")
    out = nc.dram_tensor("out", [SLOTS, L, D], F32, kind="ExternalOutput")

    order = sorted(range(SLOTS), key=lambda x: -Ks[x])

    with tile.TileContext(nc) as tc:
        with (
            tc.tile_pool(name="const", bufs=1) as constp,
            tc.tile_pool(name="io", bufs=1) as iop,
            tc.tile_pool(name="pt", bufs=6) as ptp,
            tc.tile_pool(name="fin", bufs=3) as finp,
            tc.tile_pool(name="psst", bufs=2, space="PSUM") as psst,
            tc.tile_pool(name="psoa", bufs=2, space="PSUM") as psoa,
        ):
            bias_sb = constp.tile([128, SLOTS], F32)
            nc.sync.dma_start(out=bias_sb, in_=biases[:, :])

            # preload all slot inputs upfront (SBUF easily fits them);
            # k/v first for the first slot so compute can start early,
            # q in per-block chunks so the first matmul doesn't wait on
            # the whole 512KB load
            kts, vps, qts = {}, {}, {}
            s0 = order[0]
            kts[s0] = constp.tile([128, KM * 128], BF16, tag=f"kt{s0}", name=f"kt{s0}")
            nc.sync.dma_start(out=kts[s0][:, : Ks[s0] * 128], in_=kT[s0, :, : Ks[s0] * 128])
            vps[s0] = constp.tile([128, KM, VW], BF16, tag=f"vp{s0}", name=f"vp{s0}")
            nc.sync.dma_start(
                out=vps[s0][:, : Ks[s0], :],
                in_=vp[s0, :, : Ks[s0] * VW].rearrange("p (t w) -> p t w", w=VW),
            )
            qts[s0] = constp.tile([128, L], BF16, tag=f"qt{s0}", name=f"qt{s0}")
            for qb in range(QB):
                nc.sync.dma_start(
                    out=qts[s0][:, qb * QBW : (qb + 1) * QBW],
                    in_=qT[s0, :, qb * QBW : (qb + 1) * QBW],
                )
            for s in order[1:]:
                Kv = Ks[s]
                kts[s] = constp.tile([128, KM * 128], BF16, tag=f"kt{s}", name=f"kt{s}")
                nc.sync.dma_start(out=kts[s][:, : Kv * 128], in_=kT[s, :, : Kv * 128])
                vps[s] = constp.tile([128, KM, VW], BF16, tag=f"vp{s}", name=f"vp{s}")
                nc.sync.dma_start(
                    out=vps[s][:, :Kv, :],
                    in_=vp[s, :, : Kv * VW].rearrange("p (t w) -> p t w", w=VW),
                )
                qts[s] = constp.tile([128, L], BF16, tag=f"qt{s}", name=f"qt{s}")
                nc.sync.dma_start(out=qts[s], in_=qT[s])

            # flat work list: one unit per (slot, q-block); scores for the
            # next unit's first group are emitted during the current unit's
            # second-to-last group so the Act engine never drains at unit
            # boundaries (the st pool's 2-buffer rotation already allows it)
            units = [(s, qb) for s in order for qb in range(QB)]

            def emit_scores(s, qb, g):
                j0, npair = _jgroups(Ks[s])[g]
                qs = qts[s][:, qb * QBW : (qb + 1) * QBW]
                st = psst.tile([128, npair, QBW], F32, tag="st", name="st")
                for jj in range(npair):
                    nc.tensor.matmul(
                        st[:, jj, :],
                        kts[s][:, (j0 + jj) * 128 : (j0 + jj + 1) * 128],
                        qs,
                        start=True,
                        stop=True,
                    )
                return st

            st_next = None
            for i, (s, qb) in enumerate(units):
                Kv = Ks[s]
                groups = _jgroups(Kv)
                G = len(groups)
                vp_sb = vps[s]
                sts = {0: st_next if st_next is not None else emit_scores(s, qb, 0)}
                st_next = None
                # PV in two qt-pair phases: phase A (q-tiles 0,1) accumulates
                # interleaved with the exp chain; phase B (q-tiles 2,3)
                # re-reads the kept pT tiles afterwards.  Each o_ps bank
                # holds one q-tile (the PE cannot interleave two accumulation
                # regions within one bank); col 128 accumulates l via the
                # ones column of vp.  Double-buffered so the next unit's PV
                # never waits on this unit's finish.
                o_pa = psoa.tile([128, 2, 512], F32, tag="o_ps", name="o_pa")
                pTs = []

                for g, (j0, npair) in enumerate(groups):
                    st = sts[g]
                    last = j0 + npair == Kv
                    pT = ptp.tile([128, npair, QBW], BF16, tag="pT")
                    pTs.append(pT)
                    nc.scalar.activation(
                        pT,
                        st[:, :npair, :],
                        EXPF,
                        bias=(bias_sb[:, s : s + 1] if last else 0.0),
                        scale=INV_SQRT_D,
                    )
                    if g + 1 < G:
                        sts[g + 1] = emit_scores(s, qb, g + 1)
                    if g == G - 2 and i + 1 < len(units):
                        st_next = emit_scores(*units[i + 1], 0)
                    for jj in range(npair):
                        j = j0 + jj
                        for qt in (0, 1):
                            nc.tensor.matmul(
                                o_pa[:, qt, : D + 1],
                                pT[:, jj, qt * 128 : (qt + 1) * 128],
                                vp_sb[:, j, : D + 1],
                                start=(j == 0),
                                stop=(j == Kv - 1),
                                skip_group_check=True,
                            )

                o_pb = psoa.tile([128, 2, 512], F32, tag="o_ps", name="o_pb")
                for g, (j0, npair) in enumerate(groups):
                    pT = pTs[g]
                    for jj in range(npair):
                        j = j0 + jj
                        for qt in (2, 3):
                            nc.tensor.matmul(
                                o_pb[:, qt - 2, : D + 1],
                                pT[:, jj, qt * 128 : (qt + 1) * 128],
                                vp_sb[:, j, : D + 1],
                                start=(j == 0),
                                stop=(j == Kv - 1),
                                skip_group_check=True,
                            )

                # per-phase finish: batched reciprocal + per-qt scalar mul;
                # phase A's finish overlaps phase B's PV stream
                lrec = finp.tile([128, QTB], F32, tag="lrec")
                o_sb = finp.tile([128, QTB, 128], F32, tag="o_sb")
                for ph, o_ph in ((0, o_pa), (1, o_pb)):
                    nc.vector.reciprocal(
                        lrec[:, 2 * ph : 2 * ph + 2], o_ph[:, :, D]
                    )
                    for h in (0, 1):
                        qt = 2 * ph + h
                        nc.vector.tensor_scalar_mul(
                            o_sb[:, qt, :],
                            o_ph[:, h, :D],
                            lrec[:, qt : qt + 1],
                        )
                nc.sync.dma_start(
                    out=out[s].rearrange("(b t p) d -> p b t d", p=128, t=QTB)[:, qb],
                    in_=o_sb,
                )
    nc.compile()
    return nc


def _get_program(K0: int, K1: int):
    key = (K0, K1)
    if key not in _cache:
        _cache[key] = _build(K0, K1)
    return _cache[key]


def _run(q, k, v, valid_lens, trace=False):
    q = np.asarray(q, dtype=np.float32)
    k = np.asarray(k, dtype=np.float32)
    v = np.asarray(v, dtype=np.float32)
    vl = np.asarray(valid_lens).astype(np.int64)
    K0 = int(max(1, -(-vl[0] // 128)))
    K1 = int(max(1, -(-vl[1] // 128)))
    KM = max(K0, K1)
    nc = _get_program(K0, K1)

    # per-slot mask bias column: 0 for valid positions in the last key tile,
    # -1e9 beyond valid_len
    biases = np.zeros((128, SLOTS), dtype=np.float32)
    Ks = [K0, K0, K1, K1]
    bs = [0, 0, 1, 1]
    pos = np.arange(128)
    for s in range(SLOTS):
        rem = int(vl[bs[s]]) - (Ks[s] - 1) * 128
        biases[:, s] = np.where(pos < rem, 0.0, np.float32(NEG))

    # host-side prep: [B,H,L,D] fp32 -> per-slot transposed bf16 images
    qb16 = q.astype(NPBF16)  # [B,H,L,D]
    kb16 = k.astype(NPBF16)
    vb16 = v.astype(NPBF16)

    in_maps = []
    for c in range(NCORES):
        h0, h1 = 2 * c, 2 * c + 1
        bh = [(0, h0), (0, h1), (1, h0), (1, h1)]
        qTs = np.empty((SLOTS, 128, L), dtype=NPBF16)
        kTs = np.zeros((SLOTS, 128, KM * 128), dtype=NPBF16)
        vps = np.zeros((SLOTS, 128, KM * VW), dtype=NPBF16)
        for s, (b, h) in enumerate(bh):
            qTs[s] = qb16[b, h].T
            Kv = Ks[s]
            kTs[s, :, : Kv * 128] = kb16[b, h, : Kv * 128].T
            # v SBUF image: [p, t, w]: w<128 -> v[t*128+p, w]; w==128 -> 1
            vt = np.zeros((128, Kv, VW), dtype=NPBF16)
            vt[:, :, :128] = vb16[b, h, : Kv * 128].reshape(Kv, 128, 128).transpose(
                1, 0, 2
            )
            vt[:, :, 128] = NPBF16.type(1.0)
            vps[s, :, : Kv * VW] = vt.reshape(128, Kv * VW)
        in_maps.append(
            {
                "qT": np.ascontiguousarray(qTs),
                "kT": np.ascontiguousarray(kTs),
                "vp": np.ascontiguousarray(vps),
                "biases": biases,
            }
        )

    try:
        res = run_bass_kernel_spmd(
            nc, in_maps, core_ids=list(range(NCORES)), trace=trace
        )
    except Exception:
        # transient device wedges (NRT_EXEC_UNIT_UNRECOVERABLE) have been
        # observed to clear on retry
        res = run_bass_kernel_spmd(
            nc, in_maps, core_ids=list(range(NCORES)), trace=trace
        )

    outp = np.empty((B, H, L, D), dtype=np.float32)
    for c in range(NCORES):
        o = res.results[c]["out"]
        h0, h1 = 2 * c, 2 * c + 1
        outp[0, h0] = o[0]
        outp[0, h1] = o[1]
        outp[1, h0] = o[2]
        outp[1, h1] = o[3]
    return outp, res


def kernel(q, k, v, valid_lens):
    outp, _ = _run(q, k, v, valid_lens, trace=False)
    return outp


# revision 16
# speedup vs baseline: 1.5493x; 1.0221x over previous
"""Masked dot-product attention (B=2,H=16,L=2048,D=128) on 8 trn2 NeuronCores.

Strategy (v2):
  - Shard batch*heads: core c handles (b=0,h=2c),(0,2c+1),(1,2c),(1,2c+1)
    -> 4 slots, so every core carries one K0-slot pair and one K1-slot pair
    (balanced work).
  - Host pre-transposes q and k per slot into [D, L] / [D, Kv*128] bf16 and
    pre-permutes v into its SBUF image [128, Kv, 130] bf16 with a ones column
    at d=128 -> zero on-device transposes, fully contiguous DMAs.
  - Scores: S^T[k, q] = matmul(lhsT=kT_j, rhs=qT-block) in bf16
    (1 cycle/row).  Masking is a per-partition bias on the exp of the last
    key tile only.
  - exp fused into PSUM->SBUF eviction on the Act engine with
    scale=1/sqrt(D), j-pairs share one instruction; pT evicted as bf16.
  - PV: O[q, d] computed in natural layout via
    matmul(out[128q, 129], lhsT=pT[:, jj, qtile], rhs=[V_j | ones]);
    the appended ones column accumulates the softmax denominator l per
    q-partition for free (no [1,N] l-matmul, no transposes of O or l).
  - Finish per q-block: DVE reciprocal of the l column + 4 per-partition
    scalar muls, then one contiguous 256KB DMA of the fp32 output.
  - Software pipelining: scores for group g+1 are emitted before PV of
    group g so the in-order PE queue never head-of-line blocks on the Act
    engine; st/pT/o_ps pools are double-buffered (8 PSUM banks exactly).
"""

import math

import numpy as np

try:
    import concourse.bass as bass
except ImportError:  # pragma: no cover
    import sys

    sys.path.append("/opt/trn_rl_repo")
    import concourse.bass as bass

import ml_dtypes
import concourse.mybir as mybir
import concourse.tile as tile
from concourse import bacc
from concourse.bass_utils import run_bass_kernel_spmd

B, H, L, D = 2, 16, 2048, 128
NCORES = 8
HPC = H // NCORES  # heads per core per batch
SLOTS = B * HPC  # bh slots per core
NEG = -1e9
INV_SQRT_D = 1.0 / math.sqrt(D)
F32 = mybir.dt.float32
BF16 = mybir.dt.bfloat16
QB = 4  # q blocks per slot
QBW = L // QB  # 512 q per block
QTB = QBW // 128  # 4 q tiles per block
VW = 130  # v tile width: 128 d + ones col + pad
EXPF = mybir.ActivationFunctionType.Exp
NPBF16 = np.dtype(ml_dtypes.bfloat16)

_cache: dict = {}


def _jgroups(Kv):
    """j in pairs, the last j always alone (it takes the mask bias)."""
    out = []
    j = 0
    while j < Kv - 1:
        n = 2 if j + 2 <= Kv - 1 else 1
        out.append((j, n))
        j += n
    out.append((Kv - 1, 1))
    return out


def _build(K0: int, K1: int):
    """Build+compile the per-core program for K0/K1 valid key tiles."""
    Ks = [K0, K0, K1, K1]
    KM = max(K0, K1)
    nc = bacc.Bacc("TRN2", target_bir_lowering=False, debug=False, num_devices=NCORES)
    # per-slot packed input image: [kT (KM*128) | vp (KM*130) | qT (2048)]
    KVW = KM * 258
    kvq = nc.dram_tensor("kvq", [SLOTS, 128, KVW + L], BF16, kind="ExternalInput")
    biases = nc.dram_tensor("biases", [128, SLOTS], F32, kind="ExternalInput")
    out = nc.dram_tensor("out", [SLOTS, L, D], F32, kind="ExternalOutput")

    order = sorted(range(SLOTS), key=lambda x: -Ks[x])

    with tile.TileContext(nc) as tc:
        with (
            tc.tile_pool(name="const", bufs=1) as constp,
            tc.tile_pool(name="io", bufs=1) as iop,
            tc.tile_pool(name="pt", bufs=6) as ptp,
            tc.tile_pool(name="fin", bufs=3) as finp,
            tc.tile_pool(name="psst", bufs=2, space="PSUM") as psst,
            tc.tile_pool(name="psoa", bufs=2, space="PSUM") as psoa,
        ):
            # one packed DMA per slot (slot0 split so the first scores and
            # the bias-consuming exp can start before the bulk arrives)
            kvqs = {}
            s0 = order[0]
            kvqs[s0] = constp.tile([128, KVW + L], BF16, tag=f"kvq{s0}", name=f"kvq{s0}")
            nc.sync.dma_start(
                out=kvqs[s0][:, : KVW + QBW], in_=kvq[s0, :, : KVW + QBW]
            )
            bias_sb = constp.tile([128, SLOTS], F32)
            nc.sync.dma_start(out=bias_sb, in_=biases[:, :])
            nc.sync.dma_start(
                out=kvqs[s0][:, KVW + QBW :], in_=kvq[s0, :, KVW + QBW :]
            )
            for s in order[1:]:
                kvqs[s] = constp.tile(
                    [128, KVW + L], BF16, tag=f"kvq{s}", name=f"kvq{s}"
                )
                nc.sync.dma_start(out=kvqs[s], in_=kvq[s])

            def kt_view(s, j):
                return kvqs[s][:, j * 128 : (j + 1) * 128]

            def vp_view(s, j):
                return kvqs[s][:, KM * 128 + j * VW : KM * 128 + j * VW + D + 1]

            def q_view(s, qb):
                return kvqs[s][:, KVW + qb * QBW : KVW + (qb + 1) * QBW]

            # flat work list: one unit per (slot, q-block); scores for the
            # next unit's first group are emitted during the current unit's
            # second-to-last group so the Act engine never drains at unit
            # boundaries (the st pool's 2-buffer rotation already allows it)
            units = [(s, qb) for s in order for qb in range(QB)]

            def emit_scores(s, qb, g):
                j0, npair = _jgroups(Ks[s])[g]
                qs = q_view(s, qb)
                st = psst.tile([128, npair, QBW], F32, tag="st", name="st")
                for jj in range(npair):
                    nc.tensor.matmul(
                        st[:, jj, :],
                        kt_view(s, j0 + jj),
                        qs,
                        start=True,
                        stop=True,
                    )
                return st

            st_next = None
            o_sb2 = None
            for i, (s, qb) in enumerate(units):
                Kv = Ks[s]
                groups = _jgroups(Kv)
                G = len(groups)
                sts = {0: st_next if st_next is not None else emit_scores(s, qb, 0)}
                st_next = None
                # PV in two qt-pair phases: phase A (q-tiles 0,1) accumulates
                # interleaved with the exp chain; phase B (q-tiles 2,3)
                # re-reads the kept pT tiles afterwards.  Each o_ps bank
                # holds one q-tile (the PE cannot interleave two accumulation
                # regions within one bank); col 128 accumulates l via the
                # ones column of vp.  Double-buffered so the next unit's PV
                # never waits on this unit's finish.
                o_pa = psoa.tile([128, 2, 512], F32, tag="o_ps", name="o_pa")
                pTs = []

                for g, (j0, npair) in enumerate(groups):
                    st = sts[g]
                    last = j0 + npair == Kv
                    pT = ptp.tile([128, npair, QBW], BF16, tag="pT")
                    pTs.append(pT)
                    nc.scalar.activation(
                        pT,
                        st[:, :npair, :],
                        EXPF,
                        bias=(bias_sb[:, s : s + 1] if last else 0.0),
                        scale=INV_SQRT_D,
                    )
                    if g + 1 < G:
                        sts[g + 1] = emit_scores(s, qb, g + 1)
                    if g == G - 2 and i + 1 < len(units):
                        st_next = emit_scores(*units[i + 1], 0)
                    for jj in range(npair):
                        j = j0 + jj
                        for qt in (0, 1):
                            nc.tensor.matmul(
                                o_pa[:, qt, : D + 1],
                                pT[:, jj, qt * 128 : (qt + 1) * 128],
                                vp_view(s, j),
                                start=(j == 0),
                                stop=(j == Kv - 1),
                                skip_group_check=True,
                            )

                o_pb = psoa.tile([128, 2, 512], F32, tag="o_ps", name="o_pb")
                for g, (j0, npair) in enumerate(groups):
                    pT = pTs[g]
                    for jj in range(npair):
                        j = j0 + jj
                        for qt in (2, 3):
                            nc.tensor.matmul(
                                o_pb[:, qt - 2, : D + 1],
                                pT[:, jj, qt * 128 : (qt + 1) * 128],
                                vp_view(s, j),
                                start=(j == 0),
                                stop=(j == Kv - 1),
                                skip_group_check=True,
                            )

                # per-phase finish: batched reciprocal + per-qt scalar mul;
                # phase A's finish overlaps phase B's PV stream.  Output is
                # staged per qb-PAIR so only one out-DMA per two units.
                if qb % 2 == 0:
                    o_sb2 = finp.tile([128, 2, QTB, 128], F32, tag="o_sb2", bufs=2)
                lrec = finp.tile([128, QTB], F32, tag="lrec")
                for ph, o_ph in ((0, o_pa), (1, o_pb)):
                    nc.vector.reciprocal(
                        lrec[:, 2 * ph : 2 * ph + 2], o_ph[:, :, D]
                    )
                    for h in (0, 1):
                        qt = 2 * ph + h
                        nc.vector.tensor_scalar_mul(
                            o_sb2[:, qb % 2, qt, :],
                            o_ph[:, h, :D],
                            lrec[:, qt : qt + 1],
                        )
                if qb % 2 == 1:
                    nc.sync.dma_start(
                        out=out[s].rearrange("(b t p) d -> p b t d", p=128, t=QTB)[
                            :, qb - 1 : qb + 1
                        ],
                        in_=o_sb2,
                    )
    nc.compile()
    return nc


def _get_program(K0: int, K1: int):
    key = (K0, K1)
    if key not in _cache:
        _cache[key] = _build(K0, K1)
    return _cache[key]


def _run(q, k, v, valid_lens, trace=False):
    q = np.asarray(q, dtype=np.float32)
    k = np.asarray(k, dtype=np.float32)
    v = np.asarray(v, dtype=np.float32)
    vl = np.asarray(valid_lens).astype(np.int64)
    K0 = int(max(1, -(-vl[0] // 128)))
    K1 = int(max(1, -(-vl[1] // 128)))
    KM = max(K0, K1)
    nc = _get_program(K0, K1)

    # per-slot mask bias column: 0 for valid positions in the last key tile,
    # -1e9 beyond valid_len
    biases = np.zeros((128, SLOTS), dtype=np.float32)
    Ks = [K0, K0, K1, K1]
    bs = [0, 0, 1, 1]
    pos = np.arange(128)
    for s in range(SLOTS):
        rem = int(vl[bs[s]]) - (Ks[s] - 1) * 128
        biases[:, s] = np.where(pos < rem, 0.0, np.float32(NEG))

    # host-side prep: [B,H,L,D] fp32 -> per-slot transposed bf16 images
    qb16 = q.astype(NPBF16)  # [B,H,L,D]
    kb16 = k.astype(NPBF16)
    vb16 = v.astype(NPBF16)

    KVW = KM * 258
    in_maps = []
    for c in range(NCORES):
        h0, h1 = 2 * c, 2 * c + 1
        bh = [(0, h0), (0, h1), (1, h0), (1, h1)]
        kvqs = np.zeros((SLOTS, 128, KVW + L), dtype=NPBF16)
        for s, (b, h) in enumerate(bh):
            Kv = Ks[s]
            # [kT image | vp image (with ones col) | qT image]
            kvqs[s, :, : Kv * 128] = kb16[b, h, : Kv * 128].T
            vt = np.zeros((128, Kv, VW), dtype=NPBF16)
            vt[:, :, :128] = vb16[b, h, : Kv * 128].reshape(Kv, 128, 128).transpose(
                1, 0, 2
            )
            vt[:, :, 128] = NPBF16.type(1.0)
            kvqs[s, :, KM * 128 : KM * 128 + Kv * VW] = vt.reshape(128, Kv * VW)
            kvqs[s, :, KVW:] = qb16[b, h].T
        in_maps.append(
            {
                "kvq": np.ascontiguousarray(kvqs),
                "biases": biases,
            }
        )

    try:
        res = run_bass_kernel_spmd(
            nc, in_maps, core_ids=list(range(NCORES)), trace=trace
        )
    except Exception:
        # transient device wedges (NRT_EXEC_UNIT_UNRECOVERABLE) have been
        # observed to clear on retry
        res = run_bass_kernel_spmd(
            nc, in_maps, core_ids=list(range(NCORES)), trace=trace
        )

    outp = np.empty((B, H, L, D), dtype=np.float32)
    for c in range(NCORES):
        o = res.results[c]["out"]
        h0, h1 = 2 * c, 2 * c + 1
        outp[0, h0] = o[0]
        outp[0, h1] = o[1]
        outp[1, h0] = o[2]
        outp[1, h1] = o[3]
    return outp, res


def kernel(q, k, v, valid_lens):
    outp, _ = _run(q, k, v, valid_lens, trace=False)
    return outp
